# revision 1
# baseline (speedup 1.0000x reference)
"""Trainium2 Bass kernel for nn_MinibatchTwoBranchGNN.

Two-branch 2-layer GraphSAGE with index-permuted second branch and mixing.
Strategy:
  - Shard by destination (root) across 8 cores. N1=40000 roots are split into
    a "head" part (first 10000, which are the layer-1 roots) and a "tail",
    each sharded evenly so that every core owns exactly the slice of x_mix1
    it needs for layer 1 (no exchange of roots needed).
  - Segment-sum via one-hot-indicator matmuls accumulated in PSUM; message
    gathers via dma_gather (int16 indices, tables chunked <32768 rows); the
    x0 table is padded host-side to 192 cols with a ones column at col 128 so
    segment COUNTS fall out of the same matmul for free.
  - x1 (layer-0 output, needed as layer-1 messages by all cores) is
    exchanged with chunked AllGather collectives (bf16), overlapped with
    remaining layer-0 compute.
  - All dense math f32; only the x1 exchange/messages are bf16.

Self-contained: hardcodes shapes/sharding for this problem instance.
"""
import numpy as np

# ----- problem constants (hardcoded per contract) -----
N0, N1, N2 = 120000, 40000, 10000
E0, E1 = 600000, 150000
DIN, DH, NC_CLS = 128, 256, 47
NCORES = 8

# virtual root spaces (pad to multiples of 128*NCORES)
HEADV = 10240            # virtual head roots (covers N2=10000)
TAILV = 30720            # virtual tail roots (covers 30000)
HPC = HEADV // NCORES    # 1280 head roots per core (10 blocks)
TPC = TAILV // NCORES    # 3840 tail roots per core (30 blocks)
RPC = HPC + TPC          # 5120 roots per core (40 blocks)
NBLK = RPC // 128        # 40 blocks per core
NHB = HPC // 128         # 10 head blocks
X0CH = 4                 # x0 table chunks (30000 rows each, <32768)
X0CHROWS = 30000
DEXT = 192               # x0ext row: 128 feats + ones col + pad (768B, %256==0)
X1V = NCORES * RPC       # 40960 rows in allgathered x1
X1CH = 2                 # x1 table chunks (20480 rows each)
X1CHROWS = X1V // X1CH
AG_CHUNKS = 2            # allgather in 2 pieces (overlap with phase A)

TRACE = False
DEBUG_DUMPS = False
LAST_EXEC_NS = None
DEBUG = {}


def _v_of_real(r):
    """real N1 root id -> virtual id"""
    return np.where(r < N2, r, r + (HEADV - N2))


def _core_block_of_virtual(v):
    """virtual root id -> (core, local block, in-block pos)"""
    is_head = v < HEADV
    core = np.where(is_head, v // HPC, (v - HEADV) // TPC)
    loc = np.where(is_head, v - core * HPC, HPC + (v - HEADV) - core * TPC)
    return core, loc


def _x1pos_of_virtual(v):
    """virtual root id -> row position in allgathered x1 layout.

    AllGather chunk c concatenates cores' rows [c*HPC_chunk ...]; layout:
    chunk-major then core-major then row. Per-core rows are in local order
    (block-major). Chunk c covers local rows [c*RPC/AG_CHUNKS, ...).
    """
    core, loc = _core_block_of_virtual(v)
    rows_per_chunk = RPC // AG_CHUNKS
    c = loc // rows_per_chunk
    within = loc - c * rows_per_chunk
    return (c * NCORES + core) * rows_per_chunk + within


def _pack_idx16(idx_stream):
    """int16 idx stream (len multiple of 16) -> [128, len/16] wrapped layout."""
    n = len(idx_stream)
    assert n % 16 == 0
    p = idx_stream.reshape(n // 16, 16).T.astype(np.int16)  # [16, n/16]
    return np.tile(p, (8, 1))


def _bucket_edges(src, dst_virtual, n_chunks, chunk_rows, tiles_tab=None):
    """Bucket edges by (core, block, chunk); pad each bucket to tiles*128.

    Returns per-core dict with:
      sp: int16 chunk-local src stream, dl: f32 in-block dst stream (-1 pad)
      and the uniform tiles table tiles_tab[(block, chunk)] (max over cores).
    """
    core, loc = _core_block_of_virtual(dst_virtual)
    blk = loc // 128
    inb = loc % 128
    ch = src // chunk_rows
    order = np.lexsort((src, ch, blk, core))
    core_s, blk_s, inb_s, ch_s, src_s = (core[order], blk[order], inb[order],
                                         ch[order], src[order])
    # counts per (core, block, chunk)
    key = (core_s * NBLK + blk_s) * n_chunks + ch_s
    cnt = np.bincount(key, minlength=NCORES * NBLK * n_chunks).reshape(
        NCORES, NBLK, n_chunks)
    if tiles_tab is None:
        tiles_tab = np.maximum(1, np.ceil(cnt.max(axis=0) / 128).astype(np.int64))
    assert (cnt <= tiles_tab[None] * 128).all(), "tile table overflow"
    boundaries = np.cumsum(cnt.reshape(NCORES, -1), axis=1)
    out = []
    slots_per_bc = tiles_tab * 128                      # [NBLK, n_chunks]
    total_slots = int(slots_per_bc.sum())
    # offsets of each (block, chunk) bucket in the padded stream
    bc_off = np.zeros((NBLK, n_chunks), np.int64)
    run = 0
    for b in range(NBLK):
        for c in range(n_chunks):
            bc_off[b, c] = run
            run += int(slots_per_bc[b, c])
    for k in range(NCORES):
        sp = np.zeros(total_slots, np.int64)
        dl = np.full(total_slots, -1.0, np.float32)
        sel = core_s == k
        bk, cs, ib, ss = blk_s[sel], ch_s[sel], inb_s[sel], src_s[sel]
        key2 = bk * n_chunks + cs
        # position within bucket
        ord2 = np.argsort(key2, kind="stable")
        bk, cs, ib, ss = bk[ord2], cs[ord2], ib[ord2], ss[ord2]
        kcnt = np.bincount(key2, minlength=NBLK * n_chunks)
        starts = np.concatenate([[0], np.cumsum(kcnt)[:-1]])
        within = np.arange(len(bk)) - starts[bk * n_chunks + cs]
        pos = bc_off[bk, cs] + within
        sp[pos] = ss - cs * chunk_rows
        dl[pos] = ib.astype(np.float32)
        out.append((sp.astype(np.int16), dl))
    return out, tiles_tab, bc_off


def kernel(x0, src0, dst0, src1, dst1, srcb0, dstb0, srcb1, dstb1,
           idx, mix_ratio, Wl0, Wr0, b0, Wl1, Wr1, b1, Wlin, blin):
    global LAST_EXEC_NS
    import concourse.bacc as bacc
    import concourse.bass as bass
    import concourse.mybir as mybir
    from concourse.tile import TileContext
    from concourse import bass_utils

    f32 = mybir.dt.float32
    bf16 = mybir.dt.bfloat16
    i32 = mybir.dt.int32
    i16 = mybir.dt.int16

    x0 = np.asarray(x0, np.float32)
    idx = np.asarray(idx, np.int64)
    m = float(np.asarray(mix_ratio))
    src0 = np.asarray(src0, np.int64); dst0 = np.asarray(dst0, np.int64)
    src1 = np.asarray(src1, np.int64); dst1 = np.asarray(dst1, np.int64)
    srcb0 = np.asarray(srcb0, np.int64); dstb0 = np.asarray(dstb0, np.int64)
    srcb1 = np.asarray(srcb1, np.int64); dstb1 = np.asarray(dstb1, np.int64)

    # ---------------- host prep ----------------
    x0ext = np.empty((N0, DEXT), np.float32)
    x0ext[:, :DIN] = x0
    x0ext[:, DIN] = 1.0
    x0ext[:, DIN + 1:] = 0.0

    # graph A (layer 0): main uses (src0, dst0); b uses (idx[srcb0], dstb0)
    idxc0 = idx[srcb0]
    vd0 = _v_of_real(dst0)
    vdb0 = _v_of_real(dstb0)
    bA_main, tilesA, bcoffA = _bucket_edges(src0, vd0, X0CH, X0CHROWS)
    bA_b, tilesAb, bcoffAb = _bucket_edges(idxc0, vdb0, X0CH, X0CHROWS)

    # graph B (layer 1): tables are allgathered x1 positions
    idxc1 = idx[:N1][srcb1]
    p1 = _x1pos_of_virtual(_v_of_real(src1))
    pb1 = _x1pos_of_virtual(_v_of_real(idxc1))
    # destinations: N2 roots are the head part, virtual id = dst (<10000)
    bB_main, tilesB, bcoffB = _bucket_edges(p1, dst1, X1CH, X1CHROWS)
    bB_b, tilesBb, bcoffBb = _bucket_edges(pb1, dstb1, X1CH, X1CHROWS)

    # per-core root data
    x0roots = np.zeros((NCORES, RPC, DIN), np.float32)
    idxv = np.zeros((NCORES, RPC), np.int32)
    for k in range(NCORES):
        hr0, hr1 = k * HPC, min((k + 1) * HPC, N2)
        if hr1 > hr0:
            n = hr1 - hr0
            x0roots[k, :n] = x0[hr0:hr1]
            idxv[k, :n] = idx[hr0:hr1]
        tr0 = N2 + k * TPC
        tr1 = min(N2 + (k + 1) * TPC, N1)
        if tr1 > tr0:
            n = tr1 - tr0
            x0roots[k, HPC:HPC + n] = x0[tr0:tr1]
            idxv[k, HPC:HPC + n] = idx[tr0:tr1]
    # idxv layout for per-block [128,1] indirect gathers: [128, NBLK]
    idxv_t = idxv.reshape(NCORES, NBLK, 128).transpose(0, 2, 1).copy()

    # streams: pack sp (int16) and dl (f32) as [128, S] tiles-by-column
    def _stream_tiles(sp, dl):
        S = len(sp) // 128
        sp2 = _pack_idx16(sp)
        dl2 = dl.reshape(S, 128).T.copy()
        return sp2, dl2

    spA = []; dlA = []; spAb = []; dlAb = []
    spB = []; dlB = []; spBb = []; dlBb = []
    for k in range(NCORES):
        s, d = _stream_tiles(*bA_main[k]); spA.append(s); dlA.append(d)
        s, d = _stream_tiles(*bA_b[k]); spAb.append(s); dlAb.append(d)
        s, d = _stream_tiles(*bB_main[k]); spB.append(s); dlB.append(d)
        s, d = _stream_tiles(*bB_b[k]); spBb.append(s); dlBb.append(d)

    iota_np = np.tile(np.arange(128, dtype=np.float32), (128, 1))
    mA = np.full((128, 1), m, np.float32)
    mB = np.full((128, 1), 1.0 - m, np.float32)
    b0bc = np.tile(np.asarray(b0, np.float32), (128, 1))
    b1bc = np.tile(np.asarray(b1, np.float32), (128, 1))
    blbc = np.zeros((128, 48), np.float32)
    blbc[:, :NC_CLS] = np.asarray(blin, np.float32)[None, :]

    Wl0 = np.asarray(Wl0, np.float32); Wr0 = np.asarray(Wr0, np.float32)
    Wl1 = np.asarray(Wl1, np.float32); Wr1 = np.asarray(Wr1, np.float32)
    Wlin_np = np.asarray(Wlin, np.float32)
    Wlin_pad = np.zeros((DH, 48), np.float32)
    Wlin_pad[:, :NC_CLS] = Wlin_np

    # ---------------- bass program ----------------
    nc = bacc.Bacc("TRN2", target_bir_lowering=False, debug=False,
                   num_devices=NCORES)

    t_x0ext = nc.dram_tensor("x0ext", [N0, DEXT], f32, kind="ExternalInput")
    t_x0roots = nc.dram_tensor("x0roots", [RPC, DIN], f32, kind="ExternalInput")
    t_idxv = nc.dram_tensor("idxv", [128, NBLK], i32, kind="ExternalInput")
    t_spA = nc.dram_tensor("spA", list(spA[0].shape), i16, kind="ExternalInput")
    t_dlA = nc.dram_tensor("dlA", list(dlA[0].shape), f32, kind="ExternalInput")
    t_spAb = nc.dram_tensor("spAb", list(spAb[0].shape), i16, kind="ExternalInput")
    t_dlAb = nc.dram_tensor("dlAb", list(dlAb[0].shape), f32, kind="ExternalInput")
    t_spB = nc.dram_tensor("spB", list(spB[0].shape), i16, kind="ExternalInput")
    t_dlB = nc.dram_tensor("dlB", list(dlB[0].shape), f32, kind="ExternalInput")
    t_spBb = nc.dram_tensor("spBb", list(spBb[0].shape), i16, kind="ExternalInput")
    t_dlBb = nc.dram_tensor("dlBb", list(dlBb[0].shape), f32, kind="ExternalInput")
    t_Wl0 = nc.dram_tensor("Wl0", [DIN, DH], f32, kind="ExternalInput")
    t_Wr0 = nc.dram_tensor("Wr0", [DIN, DH], f32, kind="ExternalInput")
    t_Wl1 = nc.dram_tensor("Wl1", [DH, DH], f32, kind="ExternalInput")
    t_Wr1 = nc.dram_tensor("Wr1", [DH, DH], f32, kind="ExternalInput")
    t_Wlin = nc.dram_tensor("Wlin", [DH, 48], f32, kind="ExternalInput")
    t_b0 = nc.dram_tensor("b0bc", [128, DH], f32, kind="ExternalInput")
    t_b1 = nc.dram_tensor("b1bc", [128, DH], f32, kind="ExternalInput")
    t_bl = nc.dram_tensor("blbc", [128, 48], f32, kind="ExternalInput")
    t_iota = nc.dram_tensor("iota", [128, 128], f32, kind="ExternalInput")
    t_mA = nc.dram_tensor("mA", [128, 1], f32, kind="ExternalInput")
    t_mB = nc.dram_tensor("mB", [128, 1], f32, kind="ExternalInput")
    t_out = nc.dram_tensor("out", [HPC, 48], f32, kind="ExternalOutput")
    if DEBUG_DUMPS:
        t_dbg_x1 = nc.dram_tensor("dbg_x1", [RPC, DH], f32, kind="ExternalOutput")
        t_dbg_xm1 = nc.dram_tensor("dbg_xm1", [HPC, DH], f32, kind="ExternalOutput")

    from concourse.masks import make_identity

    MAXTA = int(max(tilesA.max(), tilesAb.max()))
    MAXTB = int(max(tilesB.max(), tilesBb.max()))

    with TileContext(nc) as tc:
        with (
            tc.tile_pool(name="const", bufs=1) as cpool,
            tc.tile_pool(name="idxp", bufs=1) as ipool,
            tc.tile_pool(name="msg", bufs=3) as mpool,
            tc.tile_pool(name="ind", bufs=3) as indpool,
            tc.tile_pool(name="sb", bufs=3) as sb,
            tc.tile_pool(name="sbT", bufs=4) as sbT,
            tc.tile_pool(name="acc", bufs=3) as accp,
            tc.tile_pool(name="pseg", bufs=2, space="PSUM") as pseg,
            tc.tile_pool(name="ptr", bufs=2, space="PSUM") as ptr,
            tc.tile_pool(name="pd", bufs=4, space="PSUM") as pd,
            tc.tile_pool(name="dram", bufs=1, space="DRAM") as dram,
        ):
            ident = cpool.tile([128, 128], f32)
            make_identity(nc, ident[:])
            iota_t = cpool.tile([128, 128], f32)
            nc.sync.dma_start(iota_t[:], t_iota[:])
            wl0 = cpool.tile([128, DH], f32); nc.sync.dma_start(wl0[:], t_Wl0[:])
            wr0 = cpool.tile([128, DH], f32); nc.sync.dma_start(wr0[:], t_Wr0[:])
            wl1a = cpool.tile([128, DH], f32); nc.sync.dma_start(wl1a[:], t_Wl1[0:128, :])
            wl1b = cpool.tile([128, DH], f32); nc.sync.dma_start(wl1b[:], t_Wl1[128:256, :])
            wr1a = cpool.tile([128, DH], f32); nc.sync.dma_start(wr1a[:], t_Wr1[0:128, :])
            wr1b = cpool.tile([128, DH], f32); nc.sync.dma_start(wr1b[:], t_Wr1[128:256, :])
            wlina = cpool.tile([128, 48], f32); nc.sync.dma_start(wlina[:], t_Wlin[0:128, :])
            wlinb = cpool.tile([128, 48], f32); nc.sync.dma_start(wlinb[:], t_Wlin[128:256, :])
            b0t = cpool.tile([128, DH], f32); nc.sync.dma_start(b0t[:], t_b0[:])
            b1t = cpool.tile([128, DH], f32); nc.sync.dma_start(b1t[:], t_b1[:])
            blt = cpool.tile([128, 48], f32); nc.sync.dma_start(blt[:], t_bl[:])
            mAt = cpool.tile([128, 1], f32); nc.sync.dma_start(mAt[:], t_mA[:])
            mBt = cpool.tile([128, 1], f32); nc.sync.dma_start(mBt[:], t_mB[:])
            ones1bf = cpool.tile([128, 1], bf16)
            nc.vector.memset(ones1bf[:], 1.0)

            # preload all index/dst streams
            spA_t = ipool.tile(list(spA[0].shape), i16)
            nc.sync.dma_start(spA_t[:], t_spA[:])
            dlA_t = ipool.tile(list(dlA[0].shape), f32)
            nc.sync.dma_start(dlA_t[:], t_dlA[:])
            spAb_t = ipool.tile(list(spAb[0].shape), i16)
            nc.sync.dma_start(spAb_t[:], t_spAb[:])
            dlAb_t = ipool.tile(list(dlAb[0].shape), f32)
            nc.sync.dma_start(dlAb_t[:], t_dlAb[:])
            spB_t = ipool.tile(list(spB[0].shape), i16)
            nc.sync.dma_start(spB_t[:], t_spB[:])
            dlB_t = ipool.tile(list(dlB[0].shape), f32)
            nc.sync.dma_start(dlB_t[:], t_dlB[:])
            spBb_t = ipool.tile(list(spBb[0].shape), i16)
            nc.sync.dma_start(spBb_t[:], t_spBb[:])
            dlBb_t = ipool.tile(list(dlBb[0].shape), f32)
            nc.sync.dma_start(dlBb_t[:], t_dlBb[:])
            idxv_t_sb = ipool.tile([128, NBLK], i32)
            nc.sync.dma_start(idxv_t_sb[:], t_idxv[:])

            x1slice = dram.tile([RPC, DH], bf16)
            assert AG_CHUNKS == X1CH and NCORES * (RPC // AG_CHUNKS) == X1CHROWS
            x1bs = [dram.tile([X1CHROWS, DH], bf16, addr_space="Shared",
                              tag=f"x1b{c}", name=f"x1b{c}")
                    for c in range(AG_CHUNKS)]
            xm1head = dram.tile([HPC, DH], f32)

            def seg_graph_A(spbuf, dlbuf, tiles_tab, bcoff, blk):
                """segment sums for one layer-0 graph, one block.
                Returns psum [128, 132] (cols 0:128 sums, col 128 counts)."""
                ps = pseg.tile([128, DH], f32, tag="seg")
                first = True
                total_tiles = int(tiles_tab[blk].sum())
                done = 0
                for ch in range(X0CH):
                    T = int(tiles_tab[blk, ch])
                    off = int(bcoff[blk, ch])  # slot offset
                    NI = T * 128
                    msgs = mpool.tile([128, MAXTA, DEXT], f32, tag="msgsA")
                    for t0 in range(0, T, 8):
                        t1 = min(t0 + 8, T)
                        ni = (t1 - t0) * 128
                        o2 = off + t0 * 128
                        nc.gpsimd.dma_gather(
                            msgs[:, t0:t1, :],
                            t_x0ext[ch * X0CHROWS:(ch + 1) * X0CHROWS, :],
                            spbuf[:, o2 // 16:(o2 + ni) // 16],
                            ni, ni, DEXT,
                        )
                    ind = indpool.tile([128, MAXTA, 128], f32, tag="indA")
                    dsl = dlbuf[:, off // 128:(off + NI) // 128]
                    iota_b = bass.AP(iota_t[:].tensor, iota_t[:].offset,
                                     [iota_t[:].ap[0], [0, T], iota_t[:].ap[1]])
                    dl_b = bass.AP(dsl.tensor, dsl.offset,
                                   [dsl.ap[0], dsl.ap[1], [0, 128]])
                    nc.vector.tensor_tensor(out=ind[:, 0:T, :], in0=iota_b,
                                            in1=dl_b, op=mybir.AluOpType.is_equal)
                    for t in range(T):
                        done += 1
                        nc.tensor.matmul(ps[:, 0:DIN + 1], lhsT=ind[:, t, :],
                                         rhs=msgs[:, t, 0:DIN + 1],
                                         start=first, stop=(done == total_tiles))
                        first = False
                return ps

            def transpose128(src_ap, dt=f32, tag="tr"):
                tp = ptr.tile([128, 128], f32, tag="trp")
                nc.tensor.transpose(tp[:], src_ap, ident[:])
                out = sbT.tile([128, 128], dt, tag=tag)
                nc.vector.tensor_copy(out=out[:], in_=tp[:])
                return out

            # ---------------- phase A: 40 blocks ----------------
            for blk in range(NBLK):
                psA = seg_graph_A(spA_t[:], dlA_t[:], tilesA, bcoffA, blk)
                psB = seg_graph_A(spAb_t[:], dlAb_t[:], tilesAb, bcoffAb, blk)

                recA = sb.tile([128, 1], f32, tag="recA")
                nc.vector.tensor_scalar(out=recA[:], in0=psA[:, DIN:DIN + 1],
                                        scalar1=1.0, scalar2=None,
                                        op0=mybir.AluOpType.max)
                nc.vector.reciprocal(out=recA[:], in_=recA[:])
                recB = sb.tile([128, 1], f32, tag="recB")
                nc.vector.tensor_scalar(out=recB[:], in0=psB[:, DIN:DIN + 1],
                                        scalar1=1.0, scalar2=None,
                                        op0=mybir.AluOpType.max)
                nc.vector.reciprocal(out=recB[:], in_=recB[:])

                s0 = sb.tile([128, DIN], f32, tag="s0")
                nc.vector.tensor_copy(out=s0[:], in_=psA[:, 0:DIN])
                s0b = sb.tile([128, DIN], f32, tag="s0b")
                nc.vector.tensor_copy(out=s0b[:], in_=psB[:, 0:DIN])
                s0T = transpose128(s0[:], tag="s0T")
                s0bT = transpose128(s0b[:], tag="s0bT")

                # roots
                x0r = sb.tile([128, DIN], f32, tag="x0r")
                nc.sync.dma_start(x0r[:], t_x0roots[blk * 128:(blk + 1) * 128, :])
                x0i = sb.tile([128, DEXT], f32, tag="x0i")
                nc.gpsimd.indirect_dma_start(
                    out=x0i[:], out_offset=None,
                    in_=t_x0ext[:],
                    in_offset=bass.IndirectOffsetOnAxis(
                        ap=idxv_t_sb[:, blk:blk + 1], axis=0),
                )
                x0rT = transpose128(x0r[:], tag="x0rT")
                x0iT = transpose128(x0i[:, 0:DIN], tag="x0iT")
                xm0T = sbT.tile([128, 128], f32, tag="xm0T")
                nc.vector.tensor_scalar(out=xm0T[:], in0=x0rT[:], scalar1=mAt[:, 0:1],
                                        scalar2=None, op0=mybir.AluOpType.mult)
                tmpT = sbT.tile([128, 128], f32, tag="tmpT")
                nc.vector.tensor_scalar(out=tmpT[:], in0=x0iT[:], scalar1=mBt[:, 0:1],
                                        scalar2=None, op0=mybir.AluOpType.mult)
                nc.vector.tensor_tensor(out=xm0T[:], in0=xm0T[:], in1=tmpT[:],
                                        op=mybir.AluOpType.add)

                # dense matmuls
                ps1 = pd.tile([128, DH], f32, tag="d")
                nc.tensor.matmul(ps1[:], lhsT=s0T[:], rhs=wl0[:], start=True, stop=True)
                wlA = sb.tile([128, DH], f32, tag="wlA")
                nc.vector.tensor_scalar(out=wlA[:], in0=ps1[:], scalar1=recA[:, 0:1],
                                        scalar2=None, op0=mybir.AluOpType.mult)
                ps3 = pd.tile([128, DH], f32, tag="d")
                nc.tensor.matmul(ps3[:], lhsT=s0bT[:], rhs=wl0[:], start=True, stop=True)
                wlB = sb.tile([128, DH], f32, tag="wlB")
                nc.vector.tensor_scalar(out=wlB[:], in0=ps3[:], scalar1=recB[:, 0:1],
                                        scalar2=None, op0=mybir.AluOpType.mult)
                ps2 = pd.tile([128, DH], f32, tag="d")
                nc.tensor.matmul(ps2[:], lhsT=x0rT[:], rhs=wr0[:], start=True, stop=True)
                ps4 = pd.tile([128, DH], f32, tag="d")
                nc.tensor.matmul(ps4[:], lhsT=xm0T[:], rhs=wr0[:], start=True, stop=True)

                # x1 = relu(wlA + ps2 + b0)  -> bf16 store
                tx1 = accp.tile([128, DH], f32, tag="tx1")
                nc.vector.tensor_tensor(out=tx1[:], in0=wlA[:], in1=ps2[:],
                                        op=mybir.AluOpType.add)
                nc.vector.tensor_tensor(out=tx1[:], in0=tx1[:], in1=b0t[:],
                                        op=mybir.AluOpType.add)
                x1bf = accp.tile([128, DH], bf16, tag="x1bf")
                nc.scalar.activation(out=x1bf[:], in_=tx1[:],
                                     func=mybir.ActivationFunctionType.Relu)
                nc.sync.dma_start(x1slice[blk * 128:(blk + 1) * 128, :], x1bf[:])
                if DEBUG_DUMPS:
                    x1f = accp.tile([128, DH], f32, tag="x1f")
                    nc.scalar.activation(out=x1f[:], in_=tx1[:],
                                         func=mybir.ActivationFunctionType.Relu)
                    nc.sync.dma_start(t_dbg_x1[blk * 128:(blk + 1) * 128, :], x1f[:])

                # xm1 = m*relu(wlA + ps4 + b0) + (1-m)*relu(wlB + ps4 + b0)
                tA = accp.tile([128, DH], f32, tag="tA")
                nc.vector.tensor_tensor(out=tA[:], in0=wlA[:], in1=ps4[:],
                                        op=mybir.AluOpType.add)
                nc.vector.tensor_tensor(out=tA[:], in0=tA[:], in1=b0t[:],
                                        op=mybir.AluOpType.add)
                oA = accp.tile([128, DH], f32, tag="oA")
                nc.scalar.activation(out=oA[:], in_=tA[:],
                                     func=mybir.ActivationFunctionType.Relu,
                                     scale=mAt[:, 0:1])
                tB = accp.tile([128, DH], f32, tag="tB")
                nc.vector.tensor_tensor(out=tB[:], in0=wlB[:], in1=ps4[:],
                                        op=mybir.AluOpType.add)
                nc.vector.tensor_tensor(out=tB[:], in0=tB[:], in1=b0t[:],
                                        op=mybir.AluOpType.add)
                oB = accp.tile([128, DH], f32, tag="oB")
                nc.scalar.activation(out=oB[:], in_=tB[:],
                                     func=mybir.ActivationFunctionType.Relu,
                                     scale=mBt[:, 0:1])
                if blk < NHB:
                    xm1 = accp.tile([128, DH], f32, tag="xm1")
                    nc.vector.tensor_tensor(out=xm1[:], in0=oA[:], in1=oB[:],
                                            op=mybir.AluOpType.add)
                    nc.sync.dma_start(xm1head[blk * 128:(blk + 1) * 128, :], xm1[:])
                    if DEBUG_DUMPS:
                        nc.sync.dma_start(t_dbg_xm1[blk * 128:(blk + 1) * 128, :], xm1[:])

                # allgather chunk when its last block is done
                rows_per_chunk = RPC // AG_CHUNKS
                blocks_per_chunk = rows_per_chunk // 128
                if (blk + 1) % blocks_per_chunk == 0:
                    c = (blk + 1) // blocks_per_chunk - 1
                    nc.gpsimd.collective_compute(
                        "AllGather", mybir.AluOpType.bypass,
                        replica_groups=[list(range(NCORES))],
                        ins=[x1slice[c * rows_per_chunk:(c + 1) * rows_per_chunk, :]],
                        outs=[x1bs[c][:]],
                    )

            # ---------------- phase B: 10 head blocks ----------------
            def seg_graph_B(spbuf, dlbuf, tiles_tab, bcoff, blk):
                """layer-1 segment sums: psum [128, 256] + counts via extra matmul."""
                ps = pseg.tile([128, DH], f32, tag="seg")
                pc = pd.tile([128, DH], f32, tag="d")  # counts in col 0
                first = True
                total_tiles = int(tiles_tab[blk].sum())
                done = 0
                for ch in range(X1CH):
                    T = int(tiles_tab[blk, ch])
                    off = int(bcoff[blk, ch])
                    NI = T * 128
                    msgs = mpool.tile([128, MAXTB, DH], bf16, tag="msgsB")
                    for t0 in range(0, T, 8):
                        t1 = min(t0 + 8, T)
                        ni = (t1 - t0) * 128
                        o2 = off + t0 * 128
                        nc.gpsimd.dma_gather(
                            msgs[:, t0:t1, :],
                            x1bs[ch][:],
                            spbuf[:, o2 // 16:(o2 + ni) // 16],
                            ni, ni, DH,
                        )
                    ind = indpool.tile([128, MAXTB, 128], bf16, tag="indB")
                    dsl = dlbuf[:, off // 128:(off + NI) // 128]
                    iota_b = bass.AP(iota_t[:].tensor, iota_t[:].offset,
                                     [iota_t[:].ap[0], [0, T], iota_t[:].ap[1]])
                    dl_b = bass.AP(dsl.tensor, dsl.offset,
                                   [dsl.ap[0], dsl.ap[1], [0, 128]])
                    nc.vector.tensor_tensor(out=ind[:, 0:T, :], in0=iota_b,
                                            in1=dl_b, op=mybir.AluOpType.is_equal)
                    for t in range(T):
                        done += 1
                        nc.tensor.matmul(ps[:], lhsT=ind[:, t, :],
                                         rhs=msgs[:, t, :],
                                         start=first, stop=(done == total_tiles))
                        nc.tensor.matmul(pc[:, 0:1], lhsT=ind[:, t, :],
                                         rhs=ones1bf[:],
                                         start=first, stop=(done == total_tiles))
                        first = False
                return ps, pc

            for blk in range(NHB):
                psA, pcA = seg_graph_B(spB_t[:], dlB_t[:], tilesB, bcoffB, blk)
                psB, pcB = seg_graph_B(spBb_t[:], dlBb_t[:], tilesBb, bcoffBb, blk)

                recA = sb.tile([128, 1], f32, tag="recA")
                nc.vector.tensor_scalar(out=recA[:], in0=pcA[:, 0:1], scalar1=1.0,
                                        scalar2=None, op0=mybir.AluOpType.max)
                nc.vector.reciprocal(out=recA[:], in_=recA[:])
                recB = sb.tile([128, 1], f32, tag="recB")
                nc.vector.tensor_scalar(out=recB[:], in0=pcB[:, 0:1], scalar1=1.0,
                                        scalar2=None, op0=mybir.AluOpType.max)
                nc.vector.reciprocal(out=recB[:], in_=recB[:])

                s1 = sb.tile([128, DH], f32, tag="s1")
                nc.vector.tensor_copy(out=s1[:], in_=psA[:])
                s1b = sb.tile([128, DH], f32, tag="s1b")
                nc.vector.tensor_copy(out=s1b[:], in_=psB[:])
                s1Tlo = transpose128(s1[:, 0:128], tag="s1Tlo")
                s1Thi = transpose128(s1[:, 128:256], tag="s1Thi")
                s1bTlo = transpose128(s1b[:, 0:128], tag="s1bTlo")
                s1bThi = transpose128(s1b[:, 128:256], tag="s1bThi")

                xm1r = sb.tile([128, DH], f32, tag="xm1r")
                nc.sync.dma_start(xm1r[:], xm1head[blk * 128:(blk + 1) * 128, :])
                xm1Tlo = transpose128(xm1r[:, 0:128], tag="xm1Tlo")
                xm1Thi = transpose128(xm1r[:, 128:256], tag="xm1Thi")

                ps1 = pd.tile([128, DH], f32, tag="d")
                nc.tensor.matmul(ps1[:], lhsT=s1Tlo[:], rhs=wl1a[:], start=True, stop=False)
                nc.tensor.matmul(ps1[:], lhsT=s1Thi[:], rhs=wl1b[:], start=False, stop=True)
                wlA2 = sb.tile([128, DH], f32, tag="wlA2")
                nc.vector.tensor_scalar(out=wlA2[:], in0=ps1[:], scalar1=recA[:, 0:1],
                                        scalar2=None, op0=mybir.AluOpType.mult)
                ps3 = pd.tile([128, DH], f32, tag="d")
                nc.tensor.matmul(ps3[:], lhsT=s1bTlo[:], rhs=wl1a[:], start=True, stop=False)
                nc.tensor.matmul(ps3[:], lhsT=s1bThi[:], rhs=wl1b[:], start=False, stop=True)
                wlB2 = sb.tile([128, DH], f32, tag="wlB2")
                nc.vector.tensor_scalar(out=wlB2[:], in0=ps3[:], scalar1=recB[:, 0:1],
                                        scalar2=None, op0=mybir.AluOpType.mult)
                ps4 = pd.tile([128, DH], f32, tag="d")
                nc.tensor.matmul(ps4[:], lhsT=xm1Tlo[:], rhs=wr1a[:], start=True, stop=False)
                nc.tensor.matmul(ps4[:], lhsT=xm1Thi[:], rhs=wr1b[:], start=False, stop=True)

                tA = accp.tile([128, DH], f32, tag="tA")
                nc.vector.tensor_tensor(out=tA[:], in0=wlA2[:], in1=ps4[:],
                                        op=mybir.AluOpType.add)
                nc.vector.tensor_tensor(out=tA[:], in0=tA[:], in1=b1t[:],
                                        op=mybir.AluOpType.add)
                oA = accp.tile([128, DH], f32, tag="oA")
                nc.scalar.activation(out=oA[:], in_=tA[:],
                                     func=mybir.ActivationFunctionType.Relu,
                                     scale=mAt[:, 0:1])
                tB = accp.tile([128, DH], f32, tag="tB")
                nc.vector.tensor_tensor(out=tB[:], in0=wlB2[:], in1=ps4[:],
                                        op=mybir.AluOpType.add)
                nc.vector.tensor_tensor(out=tB[:], in0=tB[:], in1=b1t[:],
                                        op=mybir.AluOpType.add)
                oB = accp.tile([128, DH], f32, tag="oB")
                nc.scalar.activation(out=oB[:], in_=tB[:],
                                     func=mybir.ActivationFunctionType.Relu,
                                     scale=mBt[:, 0:1])
                xm2 = accp.tile([128, DH], f32, tag="xm2")
                nc.vector.tensor_tensor(out=xm2[:], in0=oA[:], in1=oB[:],
                                        op=mybir.AluOpType.add)

                xm2Tlo = transpose128(xm2[:, 0:128], tag="xm2Tlo")
                xm2Thi = transpose128(xm2[:, 128:256], tag="xm2Thi")
                pl = pd.tile([128, DH], f32, tag="d")
                nc.tensor.matmul(pl[:, 0:48], lhsT=xm2Tlo[:], rhs=wlina[:],
                                 start=True, stop=False)
                nc.tensor.matmul(pl[:, 0:48], lhsT=xm2Thi[:], rhs=wlinb[:],
                                 start=False, stop=True)
                logits = accp.tile([128, 48], f32, tag="logits")
                nc.vector.tensor_tensor(out=logits[:], in0=pl[:, 0:48], in1=blt[:],
                                        op=mybir.AluOpType.add)
                # log_softmax over first 47 cols
                mx = sb.tile([128, 1], f32, tag="mx")
                nc.vector.reduce_max(out=mx[:], in_=logits[:, 0:NC_CLS],
                                     axis=mybir.AxisListType.X)
                tshift = accp.tile([128, 48], f32, tag="tshift")
                nc.vector.tensor_scalar(out=tshift[:, 0:NC_CLS],
                                        in0=logits[:, 0:NC_CLS],
                                        scalar1=mx[:, 0:1], scalar2=None,
                                        op0=mybir.AluOpType.subtract)
                ex = accp.tile([128, 48], f32, tag="ex")
                nc.scalar.activation(out=ex[:, 0:NC_CLS], in_=tshift[:, 0:NC_CLS],
                                     func=mybir.ActivationFunctionType.Exp)
                sm = sb.tile([128, 1], f32, tag="sm")
                nc.vector.reduce_sum(out=sm[:], in_=ex[:, 0:NC_CLS],
                                     axis=mybir.AxisListType.X)
                lsm = sb.tile([128, 1], f32, tag="lsm")
                nc.scalar.activation(out=lsm[:], in_=sm[:],
                                     func=mybir.ActivationFunctionType.Ln)
                outt = accp.tile([128, 48], f32, tag="outt")
                nc.vector.tensor_scalar(out=outt[:, 0:NC_CLS],
                                        in0=tshift[:, 0:NC_CLS],
                                        scalar1=lsm[:, 0:1], scalar2=None,
                                        op0=mybir.AluOpType.subtract)
                nc.sync.dma_start(t_out[blk * 128:(blk + 1) * 128, :],
                                  outt[:])

    nc.compile()

    in_maps = []
    for k in range(NCORES):
        in_maps.append({
            "x0ext": x0ext, "x0roots": x0roots[k], "idxv": idxv_t[k],
            "spA": spA[k], "dlA": dlA[k], "spAb": spAb[k], "dlAb": dlAb[k],
            "spB": spB[k], "dlB": dlB[k], "spBb": spBb[k], "dlBb": dlBb[k],
            "Wl0": Wl0, "Wr0": Wr0, "Wl1": Wl1, "Wr1": Wr1, "Wlin": Wlin_pad,
            "b0bc": b0bc, "b1bc": b1bc, "blbc": blbc,
            "iota": iota_np, "mA": mA, "mB": mB,
        })

    if TRACE:
        import sys, types, contextlib  # noqa
        if "antenv.axon_hooks" not in sys.modules:
            mod = types.ModuleType("antenv.axon_hooks")
            _h = [None]
            mod.set_axon_ntff_profile_hook = lambda h: _h.__setitem__(0, h)
            mod.get_axon_ntff_profile_hook = lambda: _h[0]
            sys.modules["antenv.axon_hooks"] = mod
            try:
                from trn_agent_boot.trn_boot import _ntff_profile_via_ctypes
                mod.set_axon_ntff_profile_hook(
                    _ntff_profile_via_ctypes("/opt/axon/libaxon_pjrt.so"))
            except Exception:
                pass

    last_err = None
    for attempt in range(4):
        try:
            res = bass_utils.run_bass_kernel_spmd(
                nc, in_maps, core_ids=list(range(NCORES)), trace=TRACE)
            break
        except Exception as e:  # noqa: BLE001
            last_err = e
            import jax
            try:
                jax.clear_caches()
            except Exception:
                pass
    else:
        raise last_err

    LAST_EXEC_NS = res.exec_time_ns

    if DEBUG_DUMPS:
        DEBUG["x1"] = [res.results[k]["dbg_x1"] for k in range(NCORES)]
        DEBUG["xm1"] = [res.results[k]["dbg_xm1"] for k in range(NCORES)]
    out = np.zeros((N2, NC_CLS), np.float32)
    for k in range(NCORES):
        r0 = k * HPC
        r1 = min((k + 1) * HPC, N2)
        if r1 > r0:
            out[r0:r1] = res.results[k]["out"][:r1 - r0, :NC_CLS]
    return out



# revision 2
# speedup vs baseline: 1.1215x; 1.1215x over previous
"""Trainium2 Bass kernel for nn_MinibatchTwoBranchGNN.

Two-branch 2-layer GraphSAGE with index-permuted second branch and mixing.
Strategy:
  - Shard by destination (root) across 8 cores. N1=40000 roots are split into
    a "head" part (first 10000, which are the layer-1 roots) and a "tail",
    each sharded evenly so that every core owns exactly the slice of x_mix1
    it needs for layer 1 (no exchange of roots needed).
  - Segment-sum via one-hot-indicator matmuls accumulated in PSUM; message
    gathers via dma_gather (int16 indices, tables chunked <32768 rows); the
    x0 table is padded host-side to 192 cols with a ones column at col 128 so
    segment COUNTS fall out of the same matmul for free.
  - x1 (layer-0 output, needed as layer-1 messages by all cores) is
    exchanged with chunked AllGather collectives (bf16), overlapped with
    remaining layer-0 compute.
  - All dense math f32; only the x1 exchange/messages are bf16.

Self-contained: hardcodes shapes/sharding for this problem instance.
"""
import numpy as np

# ----- problem constants (hardcoded per contract) -----
N0, N1, N2 = 120000, 40000, 10000
E0, E1 = 600000, 150000
DIN, DH, NC_CLS = 128, 256, 47
NCORES = 8

# virtual root spaces (pad to multiples of 128*NCORES)
HEADV = 10240            # virtual head roots (covers N2=10000)
TAILV = 30720            # virtual tail roots (covers 30000)
HPC = HEADV // NCORES    # 1280 head roots per core (10 blocks)
TPC = TAILV // NCORES    # 3840 tail roots per core (30 blocks)
RPC = HPC + TPC          # 5120 roots per core (40 blocks)
NBLK = RPC // 128        # 40 blocks per core
NHB = HPC // 128         # 10 head blocks
X0CH = 4                 # x0 table chunks (30000 rows each, <32768)
X0CHROWS = 30000
DEXT = 192               # x0ext row: 128 feats + ones col + pad (768B, %256==0)
X1V = NCORES * RPC       # 40960 rows in allgathered x1
X1CH = 2                 # x1 table chunks (20480 rows each)
X1CHROWS = X1V // X1CH
AG_CHUNKS = 2            # allgather in 2 pieces (overlap with phase A)

TRACE = False
DEBUG_DUMPS = False
LAST_EXEC_NS = None
DEBUG = {}


def _v_of_real(r):
    """real N1 root id -> virtual id"""
    return np.where(r < N2, r, r + (HEADV - N2))


def _core_block_of_virtual(v):
    """virtual root id -> (core, local block, in-block pos)"""
    is_head = v < HEADV
    core = np.where(is_head, v // HPC, (v - HEADV) // TPC)
    loc = np.where(is_head, v - core * HPC, HPC + (v - HEADV) - core * TPC)
    return core, loc


def _x1pos_of_virtual(v):
    """virtual root id -> row position in allgathered x1 layout.

    AllGather chunk c concatenates cores' rows [c*HPC_chunk ...]; layout:
    chunk-major then core-major then row. Per-core rows are in local order
    (block-major). Chunk c covers local rows [c*RPC/AG_CHUNKS, ...).
    """
    core, loc = _core_block_of_virtual(v)
    rows_per_chunk = RPC // AG_CHUNKS
    c = loc // rows_per_chunk
    within = loc - c * rows_per_chunk
    return (c * NCORES + core) * rows_per_chunk + within


def _pack_idx16(idx_stream):
    """int16 idx stream (len multiple of 16) -> [128, len/16] wrapped layout."""
    n = len(idx_stream)
    assert n % 16 == 0
    p = idx_stream.reshape(n // 16, 16).T.astype(np.int16)  # [16, n/16]
    return np.tile(p, (8, 1))


def _bucket_edges(src, dst_virtual, n_chunks, chunk_rows, tiles_tab=None):
    """Bucket edges by (core, block, chunk); pad each bucket to tiles*128.

    Returns per-core dict with:
      sp: int16 chunk-local src stream, dl: f32 in-block dst stream (-1 pad)
      and the uniform tiles table tiles_tab[(block, chunk)] (max over cores).
    """
    core, loc = _core_block_of_virtual(dst_virtual)
    blk = loc // 128
    inb = loc % 128
    ch = src // chunk_rows
    order = np.lexsort((src, ch, blk, core))
    core_s, blk_s, inb_s, ch_s, src_s = (core[order], blk[order], inb[order],
                                         ch[order], src[order])
    # counts per (core, block, chunk)
    key = (core_s * NBLK + blk_s) * n_chunks + ch_s
    cnt = np.bincount(key, minlength=NCORES * NBLK * n_chunks).reshape(
        NCORES, NBLK, n_chunks)
    if tiles_tab is None:
        tiles_tab = np.ceil(cnt.max(axis=0) / 128).astype(np.int64)
    assert (cnt <= tiles_tab[None] * 128).all(), "tile table overflow"
    boundaries = np.cumsum(cnt.reshape(NCORES, -1), axis=1)
    out = []
    slots_per_bc = tiles_tab * 128                      # [NBLK, n_chunks]
    total_slots = int(slots_per_bc.sum())
    # offsets of each (block, chunk) bucket in the padded stream
    bc_off = np.zeros((NBLK, n_chunks), np.int64)
    run = 0
    for b in range(NBLK):
        for c in range(n_chunks):
            bc_off[b, c] = run
            run += int(slots_per_bc[b, c])
    for k in range(NCORES):
        sp = np.zeros(total_slots, np.int64)
        dl = np.full(total_slots, -1.0, np.float32)
        sel = core_s == k
        bk, cs, ib, ss = blk_s[sel], ch_s[sel], inb_s[sel], src_s[sel]
        key2 = bk * n_chunks + cs
        # position within bucket
        ord2 = np.argsort(key2, kind="stable")
        bk, cs, ib, ss = bk[ord2], cs[ord2], ib[ord2], ss[ord2]
        kcnt = np.bincount(key2, minlength=NBLK * n_chunks)
        starts = np.concatenate([[0], np.cumsum(kcnt)[:-1]])
        within = np.arange(len(bk)) - starts[bk * n_chunks + cs]
        pos = bc_off[bk, cs] + within
        sp[pos] = ss - cs * chunk_rows
        dl[pos] = ib.astype(np.float32)
        out.append((sp.astype(np.int16), dl))
    return out, tiles_tab, bc_off


def kernel(x0, src0, dst0, src1, dst1, srcb0, dstb0, srcb1, dstb1,
           idx, mix_ratio, Wl0, Wr0, b0, Wl1, Wr1, b1, Wlin, blin):
    global LAST_EXEC_NS
    import concourse.bacc as bacc
    import concourse.bass as bass
    import concourse.mybir as mybir
    from concourse.tile import TileContext
    from concourse import bass_utils

    f32 = mybir.dt.float32
    bf16 = mybir.dt.bfloat16
    i32 = mybir.dt.int32
    i16 = mybir.dt.int16

    x0 = np.asarray(x0, np.float32)
    idx = np.asarray(idx, np.int64)
    m = float(np.asarray(mix_ratio))
    src0 = np.asarray(src0, np.int64); dst0 = np.asarray(dst0, np.int64)
    src1 = np.asarray(src1, np.int64); dst1 = np.asarray(dst1, np.int64)
    srcb0 = np.asarray(srcb0, np.int64); dstb0 = np.asarray(dstb0, np.int64)
    srcb1 = np.asarray(srcb1, np.int64); dstb1 = np.asarray(dstb1, np.int64)

    # ---------------- host prep ----------------
    x0ext = np.empty((N0, DEXT), np.float32)
    x0ext[:, :DIN] = x0
    x0ext[:, DIN] = 1.0
    x0ext[:, DIN + 1:] = 0.0

    # graph A (layer 0): main uses (src0, dst0); b uses (idx[srcb0], dstb0)
    selb = dstb0 < N2
    idxc0 = idx[srcb0[selb]]
    vd0 = _v_of_real(dst0)
    vdb0 = dstb0[selb]          # < N2 -> virtual id == real id (head)
    bA_main, tilesA, bcoffA = _bucket_edges(src0, vd0, X0CH, X0CHROWS)
    bA_b, tilesAb, bcoffAb = _bucket_edges(idxc0, vdb0, X0CH, X0CHROWS)

    # graph B (layer 1): tables are allgathered x1 positions
    idxc1 = idx[:N1][srcb1]
    p1 = _x1pos_of_virtual(_v_of_real(src1))
    pb1 = _x1pos_of_virtual(_v_of_real(idxc1))
    # destinations: N2 roots are the head part, virtual id = dst (<10000)
    bB_main, tilesB, bcoffB = _bucket_edges(p1, dst1, X1CH, X1CHROWS)
    bB_b, tilesBb, bcoffBb = _bucket_edges(pb1, dstb1, X1CH, X1CHROWS)

    # per-core root data
    x0roots = np.zeros((NCORES, RPC, DIN), np.float32)
    idxv = np.zeros((NCORES, RPC), np.int32)
    for k in range(NCORES):
        hr0, hr1 = k * HPC, min((k + 1) * HPC, N2)
        if hr1 > hr0:
            n = hr1 - hr0
            x0roots[k, :n] = x0[hr0:hr1]
            idxv[k, :n] = idx[hr0:hr1]
        tr0 = N2 + k * TPC
        tr1 = min(N2 + (k + 1) * TPC, N1)
        if tr1 > tr0:
            n = tr1 - tr0
            x0roots[k, HPC:HPC + n] = x0[tr0:tr1]
            idxv[k, HPC:HPC + n] = idx[tr0:tr1]
    # idxv layout for per-block [128,1] indirect gathers: [128, NBLK]
    idxv_t = idxv.reshape(NCORES, NBLK, 128).transpose(0, 2, 1).copy()

    # streams: pack sp (int16) and dl (f32) as [128, S] tiles-by-column
    def _stream_tiles(sp, dl):
        S = len(sp) // 128
        sp2 = _pack_idx16(sp)
        dl2 = dl.reshape(S, 128).T.copy()
        return sp2, dl2

    spA = []; dlA = []; spAb = []; dlAb = []
    spB = []; dlB = []; spBb = []; dlBb = []
    for k in range(NCORES):
        s, d = _stream_tiles(*bA_main[k]); spA.append(s); dlA.append(d)
        s, d = _stream_tiles(*bA_b[k]); spAb.append(s); dlAb.append(d)
        s, d = _stream_tiles(*bB_main[k]); spB.append(s); dlB.append(d)
        s, d = _stream_tiles(*bB_b[k]); spBb.append(s); dlBb.append(d)

    iota_np = np.tile(np.arange(128, dtype=np.float32), (128, 1))
    mA = np.full((128, 1), m, np.float32)
    mB = np.full((128, 1), 1.0 - m, np.float32)
    b0bc = np.tile(np.asarray(b0, np.float32), (128, 1))
    b1bc = np.tile(np.asarray(b1, np.float32), (128, 1))
    blbc = np.zeros((128, 48), np.float32)
    blbc[:, :NC_CLS] = np.asarray(blin, np.float32)[None, :]

    Wl0 = np.asarray(Wl0, np.float32); Wr0 = np.asarray(Wr0, np.float32)
    Wl1 = np.asarray(Wl1, np.float32); Wr1 = np.asarray(Wr1, np.float32)
    Wlin_np = np.asarray(Wlin, np.float32)
    Wlin_pad = np.zeros((DH, 48), np.float32)
    Wlin_pad[:, :NC_CLS] = Wlin_np

    # ---------------- bass program ----------------
    nc = bacc.Bacc("TRN2", target_bir_lowering=False, debug=False,
                   num_devices=NCORES)

    t_x0ext = nc.dram_tensor("x0ext", [N0, DEXT], f32, kind="ExternalInput")
    t_x0roots = nc.dram_tensor("x0roots", [RPC, DIN], f32, kind="ExternalInput")
    t_idxv = nc.dram_tensor("idxv", [128, NBLK], i32, kind="ExternalInput")
    t_spA = nc.dram_tensor("spA", list(spA[0].shape), i16, kind="ExternalInput")
    t_dlA = nc.dram_tensor("dlA", list(dlA[0].shape), f32, kind="ExternalInput")
    t_spAb = nc.dram_tensor("spAb", list(spAb[0].shape), i16, kind="ExternalInput")
    t_dlAb = nc.dram_tensor("dlAb", list(dlAb[0].shape), f32, kind="ExternalInput")
    t_spB = nc.dram_tensor("spB", list(spB[0].shape), i16, kind="ExternalInput")
    t_dlB = nc.dram_tensor("dlB", list(dlB[0].shape), f32, kind="ExternalInput")
    t_spBb = nc.dram_tensor("spBb", list(spBb[0].shape), i16, kind="ExternalInput")
    t_dlBb = nc.dram_tensor("dlBb", list(dlBb[0].shape), f32, kind="ExternalInput")
    t_Wl0 = nc.dram_tensor("Wl0", [DIN, DH], f32, kind="ExternalInput")
    t_Wr0 = nc.dram_tensor("Wr0", [DIN, DH], f32, kind="ExternalInput")
    t_Wl1 = nc.dram_tensor("Wl1", [DH, DH], f32, kind="ExternalInput")
    t_Wr1 = nc.dram_tensor("Wr1", [DH, DH], f32, kind="ExternalInput")
    t_Wlin = nc.dram_tensor("Wlin", [DH, 48], f32, kind="ExternalInput")
    t_b0 = nc.dram_tensor("b0bc", [128, DH], f32, kind="ExternalInput")
    t_b1 = nc.dram_tensor("b1bc", [128, DH], f32, kind="ExternalInput")
    t_bl = nc.dram_tensor("blbc", [128, 48], f32, kind="ExternalInput")
    t_iota = nc.dram_tensor("iota", [128, 128], f32, kind="ExternalInput")
    t_mA = nc.dram_tensor("mA", [128, 1], f32, kind="ExternalInput")
    t_mB = nc.dram_tensor("mB", [128, 1], f32, kind="ExternalInput")
    t_out = nc.dram_tensor("out", [HPC, 48], f32, kind="ExternalOutput")
    if DEBUG_DUMPS:
        t_dbg_x1 = nc.dram_tensor("dbg_x1", [RPC, DH], f32, kind="ExternalOutput")
        t_dbg_xm1 = nc.dram_tensor("dbg_xm1", [HPC, DH], f32, kind="ExternalOutput")

    from concourse.masks import make_identity

    MAXTA = int(max(tilesA.max(), tilesAb.max()))
    MAXTB = int(max(tilesB.max(), tilesBb.max()))

    with TileContext(nc) as tc:
        with (
            tc.tile_pool(name="const", bufs=1) as cpool,
            tc.tile_pool(name="idxp", bufs=1) as ipool,
            tc.tile_pool(name="msg", bufs=3) as mpool,
            tc.tile_pool(name="ind", bufs=3) as indpool,
            tc.tile_pool(name="sb", bufs=3) as sb,
            tc.tile_pool(name="sbT", bufs=4) as sbT,
            tc.tile_pool(name="acc", bufs=3) as accp,
            tc.tile_pool(name="pseg", bufs=2, space="PSUM") as pseg,
            tc.tile_pool(name="ptr", bufs=2, space="PSUM") as ptr,
            tc.tile_pool(name="pd", bufs=4, space="PSUM") as pd,
            tc.tile_pool(name="dram", bufs=1, space="DRAM") as dram,
        ):
            ident = cpool.tile([128, 128], f32)
            make_identity(nc, ident[:])
            iota_t = cpool.tile([128, 128], f32)
            nc.sync.dma_start(iota_t[:], t_iota[:])
            wl0 = cpool.tile([128, DH], f32); nc.sync.dma_start(wl0[:], t_Wl0[:])
            wr0 = cpool.tile([128, DH], f32); nc.sync.dma_start(wr0[:], t_Wr0[:])
            wl1a = cpool.tile([128, DH], f32); nc.sync.dma_start(wl1a[:], t_Wl1[0:128, :])
            wl1b = cpool.tile([128, DH], f32); nc.sync.dma_start(wl1b[:], t_Wl1[128:256, :])
            wr1a = cpool.tile([128, DH], f32); nc.sync.dma_start(wr1a[:], t_Wr1[0:128, :])
            wr1b = cpool.tile([128, DH], f32); nc.sync.dma_start(wr1b[:], t_Wr1[128:256, :])
            wlina = cpool.tile([128, 48], f32); nc.sync.dma_start(wlina[:], t_Wlin[0:128, :])
            wlinb = cpool.tile([128, 48], f32); nc.sync.dma_start(wlinb[:], t_Wlin[128:256, :])
            b0t = cpool.tile([128, DH], f32); nc.sync.dma_start(b0t[:], t_b0[:])
            b1t = cpool.tile([128, DH], f32); nc.sync.dma_start(b1t[:], t_b1[:])
            blt = cpool.tile([128, 48], f32); nc.sync.dma_start(blt[:], t_bl[:])
            mAt = cpool.tile([128, 1], f32); nc.sync.dma_start(mAt[:], t_mA[:])
            mBt = cpool.tile([128, 1], f32); nc.sync.dma_start(mBt[:], t_mB[:])
            ones1bf = cpool.tile([128, 1], bf16)
            nc.vector.memset(ones1bf[:], 1.0)

            # preload all index/dst streams
            spA_t = ipool.tile(list(spA[0].shape), i16)
            nc.sync.dma_start(spA_t[:], t_spA[:])
            dlA_t = ipool.tile(list(dlA[0].shape), f32)
            nc.sync.dma_start(dlA_t[:], t_dlA[:])
            spAb_t = ipool.tile(list(spAb[0].shape), i16)
            nc.sync.dma_start(spAb_t[:], t_spAb[:])
            dlAb_t = ipool.tile(list(dlAb[0].shape), f32)
            nc.sync.dma_start(dlAb_t[:], t_dlAb[:])
            spB_t = ipool.tile(list(spB[0].shape), i16)
            nc.sync.dma_start(spB_t[:], t_spB[:])
            dlB_t = ipool.tile(list(dlB[0].shape), f32)
            nc.sync.dma_start(dlB_t[:], t_dlB[:])
            spBb_t = ipool.tile(list(spBb[0].shape), i16)
            nc.sync.dma_start(spBb_t[:], t_spBb[:])
            dlBb_t = ipool.tile(list(dlBb[0].shape), f32)
            nc.sync.dma_start(dlBb_t[:], t_dlBb[:])
            idxv_t_sb = ipool.tile([128, NBLK], i32)
            nc.sync.dma_start(idxv_t_sb[:], t_idxv[:])

            x1slice = dram.tile([RPC, DH], bf16)
            assert AG_CHUNKS == X1CH and NCORES * (RPC // AG_CHUNKS) == X1CHROWS
            x1bs = [dram.tile([X1CHROWS, DH], bf16, addr_space="Shared",
                              tag=f"x1b{c}", name=f"x1b{c}")
                    for c in range(AG_CHUNKS)]
            xm1head = dram.tile([HPC, DH], f32)

            def seg_graph_A(spbuf, dlbuf, tiles_tab, bcoff, blk):
                """segment sums for one layer-0 graph, one block.
                Returns psum [128, 132] (cols 0:128 sums, col 128 counts)."""
                ps = pseg.tile([128, DH], f32, tag="seg")
                first = True
                total_tiles = int(tiles_tab[blk].sum())
                assert total_tiles > 0
                done = 0
                for ch in range(X0CH):
                    T = int(tiles_tab[blk, ch])
                    if T == 0:
                        continue
                    off = int(bcoff[blk, ch])  # slot offset
                    NI = T * 128
                    msgs = mpool.tile([128, MAXTA, DEXT], f32, tag="msgsA")
                    for t0 in range(0, T, 8):
                        t1 = min(t0 + 8, T)
                        ni = (t1 - t0) * 128
                        o2 = off + t0 * 128
                        nc.gpsimd.dma_gather(
                            msgs[:, t0:t1, :],
                            t_x0ext[ch * X0CHROWS:(ch + 1) * X0CHROWS, :],
                            spbuf[:, o2 // 16:(o2 + ni) // 16],
                            ni, ni, DEXT,
                        )
                    ind = indpool.tile([128, MAXTA, 128], f32, tag="indA")
                    dsl = dlbuf[:, off // 128:(off + NI) // 128]
                    iota_b = bass.AP(iota_t[:].tensor, iota_t[:].offset,
                                     [iota_t[:].ap[0], [0, T], iota_t[:].ap[1]])
                    dl_b = bass.AP(dsl.tensor, dsl.offset,
                                   [dsl.ap[0], dsl.ap[1], [0, 128]])
                    nc.vector.tensor_tensor(out=ind[:, 0:T, :], in0=iota_b,
                                            in1=dl_b, op=mybir.AluOpType.is_equal)
                    for t in range(T):
                        done += 1
                        nc.tensor.matmul(ps[:, 0:DIN + 1], lhsT=ind[:, t, :],
                                         rhs=msgs[:, t, 0:DIN + 1],
                                         start=first, stop=(done == total_tiles))
                        first = False
                return ps

            def transpose128(src_ap, dt=f32, tag="tr"):
                tp = ptr.tile([128, 128], f32, tag="trp")
                nc.tensor.transpose(tp[:], src_ap, ident[:])
                out = sbT.tile([128, 128], dt, tag=tag)
                nc.vector.tensor_copy(out=out[:], in_=tp[:])
                return out

            # ---------------- phase A: 40 blocks ----------------
            for blk in range(NBLK):
                head = blk < NHB
                psA = seg_graph_A(spA_t[:], dlA_t[:], tilesA, bcoffA, blk)

                recA = sb.tile([128, 1], f32, tag="recA")
                nc.vector.tensor_scalar(out=recA[:], in0=psA[:, DIN:DIN + 1],
                                        scalar1=1.0, scalar2=None,
                                        op0=mybir.AluOpType.max)
                nc.vector.reciprocal(out=recA[:], in_=recA[:])

                s0 = sb.tile([128, DIN], f32, tag="s0")
                nc.vector.tensor_copy(out=s0[:], in_=psA[:, 0:DIN])
                s0T = transpose128(s0[:], tag="s0T")

                # roots
                x0r = sb.tile([128, DIN], f32, tag="x0r")
                nc.sync.dma_start(x0r[:], t_x0roots[blk * 128:(blk + 1) * 128, :])
                x0rT = transpose128(x0r[:], tag="x0rT")

                if head:
                    psB = seg_graph_A(spAb_t[:], dlAb_t[:], tilesAb, bcoffAb, blk)
                    recB = sb.tile([128, 1], f32, tag="recB")
                    nc.vector.tensor_scalar(out=recB[:], in0=psB[:, DIN:DIN + 1],
                                            scalar1=1.0, scalar2=None,
                                            op0=mybir.AluOpType.max)
                    nc.vector.reciprocal(out=recB[:], in_=recB[:])
                    s0b = sb.tile([128, DIN], f32, tag="s0b")
                    nc.vector.tensor_copy(out=s0b[:], in_=psB[:, 0:DIN])
                    s0bT = transpose128(s0b[:], tag="s0bT")

                    x0i = sb.tile([128, DEXT], f32, tag="x0i")
                    nc.gpsimd.indirect_dma_start(
                        out=x0i[:], out_offset=None,
                        in_=t_x0ext[:],
                        in_offset=bass.IndirectOffsetOnAxis(
                            ap=idxv_t_sb[:, blk:blk + 1], axis=0),
                    )
                    x0iT = transpose128(x0i[:, 0:DIN], tag="x0iT")
                    xm0T = sbT.tile([128, 128], f32, tag="xm0T")
                    nc.vector.tensor_scalar(out=xm0T[:], in0=x0rT[:], scalar1=mAt[:, 0:1],
                                            scalar2=None, op0=mybir.AluOpType.mult)
                    tmpT = sbT.tile([128, 128], f32, tag="tmpT")
                    nc.vector.tensor_scalar(out=tmpT[:], in0=x0iT[:], scalar1=mBt[:, 0:1],
                                            scalar2=None, op0=mybir.AluOpType.mult)
                    nc.vector.tensor_tensor(out=xm0T[:], in0=xm0T[:], in1=tmpT[:],
                                            op=mybir.AluOpType.add)

                # dense matmuls
                ps1 = pd.tile([128, DH], f32, tag="d")
                nc.tensor.matmul(ps1[:], lhsT=s0T[:], rhs=wl0[:], start=True, stop=True)
                wlA = sb.tile([128, DH], f32, tag="wlA")
                nc.vector.tensor_scalar(out=wlA[:], in0=ps1[:], scalar1=recA[:, 0:1],
                                        scalar2=None, op0=mybir.AluOpType.mult)
                ps2 = pd.tile([128, DH], f32, tag="d")
                nc.tensor.matmul(ps2[:], lhsT=x0rT[:], rhs=wr0[:], start=True, stop=True)
                if head:
                    ps3 = pd.tile([128, DH], f32, tag="d")
                    nc.tensor.matmul(ps3[:], lhsT=s0bT[:], rhs=wl0[:], start=True, stop=True)
                    wlB = sb.tile([128, DH], f32, tag="wlB")
                    nc.vector.tensor_scalar(out=wlB[:], in0=ps3[:], scalar1=recB[:, 0:1],
                                            scalar2=None, op0=mybir.AluOpType.mult)
                    ps4 = pd.tile([128, DH], f32, tag="d")
                    nc.tensor.matmul(ps4[:], lhsT=xm0T[:], rhs=wr0[:], start=True, stop=True)

                # x1 = relu(wlA + ps2 + b0)  -> bf16 store
                tx1 = accp.tile([128, DH], f32, tag="tx1")
                nc.vector.tensor_tensor(out=tx1[:], in0=wlA[:], in1=ps2[:],
                                        op=mybir.AluOpType.add)
                nc.vector.tensor_tensor(out=tx1[:], in0=tx1[:], in1=b0t[:],
                                        op=mybir.AluOpType.add)
                x1bf = accp.tile([128, DH], bf16, tag="x1bf")
                nc.scalar.activation(out=x1bf[:], in_=tx1[:],
                                     func=mybir.ActivationFunctionType.Relu)
                nc.sync.dma_start(x1slice[blk * 128:(blk + 1) * 128, :], x1bf[:])
                if DEBUG_DUMPS:
                    x1f = accp.tile([128, DH], f32, tag="x1f")
                    nc.scalar.activation(out=x1f[:], in_=tx1[:],
                                         func=mybir.ActivationFunctionType.Relu)
                    nc.sync.dma_start(t_dbg_x1[blk * 128:(blk + 1) * 128, :], x1f[:])

                # xm1 = m*relu(wlA + ps4 + b0) + (1-m)*relu(wlB + ps4 + b0)
                if head:
                    tA = accp.tile([128, DH], f32, tag="tA")
                    nc.vector.tensor_tensor(out=tA[:], in0=wlA[:], in1=ps4[:],
                                            op=mybir.AluOpType.add)
                    nc.vector.tensor_tensor(out=tA[:], in0=tA[:], in1=b0t[:],
                                            op=mybir.AluOpType.add)
                    oA = accp.tile([128, DH], f32, tag="oA")
                    nc.scalar.activation(out=oA[:], in_=tA[:],
                                         func=mybir.ActivationFunctionType.Relu,
                                         scale=mAt[:, 0:1])
                    tB = accp.tile([128, DH], f32, tag="tB")
                    nc.vector.tensor_tensor(out=tB[:], in0=wlB[:], in1=ps4[:],
                                            op=mybir.AluOpType.add)
                    nc.vector.tensor_tensor(out=tB[:], in0=tB[:], in1=b0t[:],
                                            op=mybir.AluOpType.add)
                    oB = accp.tile([128, DH], f32, tag="oB")
                    nc.scalar.activation(out=oB[:], in_=tB[:],
                                         func=mybir.ActivationFunctionType.Relu,
                                         scale=mBt[:, 0:1])
                    xm1 = accp.tile([128, DH], f32, tag="xm1")
                    nc.vector.tensor_tensor(out=xm1[:], in0=oA[:], in1=oB[:],
                                            op=mybir.AluOpType.add)
                    nc.sync.dma_start(xm1head[blk * 128:(blk + 1) * 128, :], xm1[:])
                    if DEBUG_DUMPS:
                        nc.sync.dma_start(t_dbg_xm1[blk * 128:(blk + 1) * 128, :], xm1[:])

                # allgather chunk when its last block is done
                rows_per_chunk = RPC // AG_CHUNKS
                blocks_per_chunk = rows_per_chunk // 128
                if (blk + 1) % blocks_per_chunk == 0:
                    c = (blk + 1) // blocks_per_chunk - 1
                    nc.gpsimd.collective_compute(
                        "AllGather", mybir.AluOpType.bypass,
                        replica_groups=[list(range(NCORES))],
                        ins=[x1slice[c * rows_per_chunk:(c + 1) * rows_per_chunk, :]],
                        outs=[x1bs[c][:]],
                    )

            # ---------------- phase B: 10 head blocks ----------------
            def seg_graph_B(spbuf, dlbuf, tiles_tab, bcoff, blk):
                """layer-1 segment sums: psum [128, 256] + counts via extra matmul."""
                ps = pseg.tile([128, DH], f32, tag="seg")
                pc = pd.tile([128, DH], f32, tag="d")  # counts in col 0
                first = True
                total_tiles = int(tiles_tab[blk].sum())
                done = 0
                for ch in range(X1CH):
                    T = int(tiles_tab[blk, ch])
                    if T == 0:
                        continue
                    off = int(bcoff[blk, ch])
                    NI = T * 128
                    msgs = mpool.tile([128, MAXTB, DH], bf16, tag="msgsB")
                    for t0 in range(0, T, 8):
                        t1 = min(t0 + 8, T)
                        ni = (t1 - t0) * 128
                        o2 = off + t0 * 128
                        nc.gpsimd.dma_gather(
                            msgs[:, t0:t1, :],
                            x1bs[ch][:],
                            spbuf[:, o2 // 16:(o2 + ni) // 16],
                            ni, ni, DH,
                        )
                    ind = indpool.tile([128, MAXTB, 128], bf16, tag="indB")
                    dsl = dlbuf[:, off // 128:(off + NI) // 128]
                    iota_b = bass.AP(iota_t[:].tensor, iota_t[:].offset,
                                     [iota_t[:].ap[0], [0, T], iota_t[:].ap[1]])
                    dl_b = bass.AP(dsl.tensor, dsl.offset,
                                   [dsl.ap[0], dsl.ap[1], [0, 128]])
                    nc.vector.tensor_tensor(out=ind[:, 0:T, :], in0=iota_b,
                                            in1=dl_b, op=mybir.AluOpType.is_equal)
                    for t in range(T):
                        done += 1
                        nc.tensor.matmul(ps[:], lhsT=ind[:, t, :],
                                         rhs=msgs[:, t, :],
                                         start=first, stop=(done == total_tiles))
                        nc.tensor.matmul(pc[:, 0:1], lhsT=ind[:, t, :],
                                         rhs=ones1bf[:],
                                         start=first, stop=(done == total_tiles))
                        first = False
                return ps, pc

            for blk in range(NHB):
                psA, pcA = seg_graph_B(spB_t[:], dlB_t[:], tilesB, bcoffB, blk)
                psB, pcB = seg_graph_B(spBb_t[:], dlBb_t[:], tilesBb, bcoffBb, blk)

                recA = sb.tile([128, 1], f32, tag="recA")
                nc.vector.tensor_scalar(out=recA[:], in0=pcA[:, 0:1], scalar1=1.0,
                                        scalar2=None, op0=mybir.AluOpType.max)
                nc.vector.reciprocal(out=recA[:], in_=recA[:])
                recB = sb.tile([128, 1], f32, tag="recB")
                nc.vector.tensor_scalar(out=recB[:], in0=pcB[:, 0:1], scalar1=1.0,
                                        scalar2=None, op0=mybir.AluOpType.max)
                nc.vector.reciprocal(out=recB[:], in_=recB[:])

                s1 = sb.tile([128, DH], f32, tag="s1")
                nc.vector.tensor_copy(out=s1[:], in_=psA[:])
                s1b = sb.tile([128, DH], f32, tag="s1b")
                nc.vector.tensor_copy(out=s1b[:], in_=psB[:])
                s1Tlo = transpose128(s1[:, 0:128], tag="s1Tlo")
                s1Thi = transpose128(s1[:, 128:256], tag="s1Thi")
                s1bTlo = transpose128(s1b[:, 0:128], tag="s1bTlo")
                s1bThi = transpose128(s1b[:, 128:256], tag="s1bThi")

                xm1r = sb.tile([128, DH], f32, tag="xm1r")
                nc.sync.dma_start(xm1r[:], xm1head[blk * 128:(blk + 1) * 128, :])
                xm1Tlo = transpose128(xm1r[:, 0:128], tag="xm1Tlo")
                xm1Thi = transpose128(xm1r[:, 128:256], tag="xm1Thi")

                ps1 = pd.tile([128, DH], f32, tag="d")
                nc.tensor.matmul(ps1[:], lhsT=s1Tlo[:], rhs=wl1a[:], start=True, stop=False)
                nc.tensor.matmul(ps1[:], lhsT=s1Thi[:], rhs=wl1b[:], start=False, stop=True)
                wlA2 = sb.tile([128, DH], f32, tag="wlA2")
                nc.vector.tensor_scalar(out=wlA2[:], in0=ps1[:], scalar1=recA[:, 0:1],
                                        scalar2=None, op0=mybir.AluOpType.mult)
                ps3 = pd.tile([128, DH], f32, tag="d")
                nc.tensor.matmul(ps3[:], lhsT=s1bTlo[:], rhs=wl1a[:], start=True, stop=False)
                nc.tensor.matmul(ps3[:], lhsT=s1bThi[:], rhs=wl1b[:], start=False, stop=True)
                wlB2 = sb.tile([128, DH], f32, tag="wlB2")
                nc.vector.tensor_scalar(out=wlB2[:], in0=ps3[:], scalar1=recB[:, 0:1],
                                        scalar2=None, op0=mybir.AluOpType.mult)
                ps4 = pd.tile([128, DH], f32, tag="d")
                nc.tensor.matmul(ps4[:], lhsT=xm1Tlo[:], rhs=wr1a[:], start=True, stop=False)
                nc.tensor.matmul(ps4[:], lhsT=xm1Thi[:], rhs=wr1b[:], start=False, stop=True)

                tA = accp.tile([128, DH], f32, tag="tA")
                nc.vector.tensor_tensor(out=tA[:], in0=wlA2[:], in1=ps4[:],
                                        op=mybir.AluOpType.add)
                nc.vector.tensor_tensor(out=tA[:], in0=tA[:], in1=b1t[:],
                                        op=mybir.AluOpType.add)
                oA = accp.tile([128, DH], f32, tag="oA")
                nc.scalar.activation(out=oA[:], in_=tA[:],
                                     func=mybir.ActivationFunctionType.Relu,
                                     scale=mAt[:, 0:1])
                tB = accp.tile([128, DH], f32, tag="tB")
                nc.vector.tensor_tensor(out=tB[:], in0=wlB2[:], in1=ps4[:],
                                        op=mybir.AluOpType.add)
                nc.vector.tensor_tensor(out=tB[:], in0=tB[:], in1=b1t[:],
                                        op=mybir.AluOpType.add)
                oB = accp.tile([128, DH], f32, tag="oB")
                nc.scalar.activation(out=oB[:], in_=tB[:],
                                     func=mybir.ActivationFunctionType.Relu,
                                     scale=mBt[:, 0:1])
                xm2 = accp.tile([128, DH], f32, tag="xm2")
                nc.vector.tensor_tensor(out=xm2[:], in0=oA[:], in1=oB[:],
                                        op=mybir.AluOpType.add)

                xm2Tlo = transpose128(xm2[:, 0:128], tag="xm2Tlo")
                xm2Thi = transpose128(xm2[:, 128:256], tag="xm2Thi")
                pl = pd.tile([128, DH], f32, tag="d")
                nc.tensor.matmul(pl[:, 0:48], lhsT=xm2Tlo[:], rhs=wlina[:],
                                 start=True, stop=False)
                nc.tensor.matmul(pl[:, 0:48], lhsT=xm2Thi[:], rhs=wlinb[:],
                                 start=False, stop=True)
                logits = accp.tile([128, 48], f32, tag="logits")
                nc.vector.tensor_tensor(out=logits[:], in0=pl[:, 0:48], in1=blt[:],
                                        op=mybir.AluOpType.add)
                # log_softmax over first 47 cols
                mx = sb.tile([128, 1], f32, tag="mx")
                nc.vector.reduce_max(out=mx[:], in_=logits[:, 0:NC_CLS],
                                     axis=mybir.AxisListType.X)
                tshift = accp.tile([128, 48], f32, tag="tshift")
                nc.vector.tensor_scalar(out=tshift[:, 0:NC_CLS],
                                        in0=logits[:, 0:NC_CLS],
                                        scalar1=mx[:, 0:1], scalar2=None,
                                        op0=mybir.AluOpType.subtract)
                ex = accp.tile([128, 48], f32, tag="ex")
                nc.scalar.activation(out=ex[:, 0:NC_CLS], in_=tshift[:, 0:NC_CLS],
                                     func=mybir.ActivationFunctionType.Exp)
                sm = sb.tile([128, 1], f32, tag="sm")
                nc.vector.reduce_sum(out=sm[:], in_=ex[:, 0:NC_CLS],
                                     axis=mybir.AxisListType.X)
                lsm = sb.tile([128, 1], f32, tag="lsm")
                nc.scalar.activation(out=lsm[:], in_=sm[:],
                                     func=mybir.ActivationFunctionType.Ln)
                outt = accp.tile([128, 48], f32, tag="outt")
                nc.vector.tensor_scalar(out=outt[:, 0:NC_CLS],
                                        in0=tshift[:, 0:NC_CLS],
                                        scalar1=lsm[:, 0:1], scalar2=None,
                                        op0=mybir.AluOpType.subtract)
                nc.sync.dma_start(t_out[blk * 128:(blk + 1) * 128, :],
                                  outt[:])

    nc.compile()

    in_maps = []
    for k in range(NCORES):
        in_maps.append({
            "x0ext": x0ext, "x0roots": x0roots[k], "idxv": idxv_t[k],
            "spA": spA[k], "dlA": dlA[k], "spAb": spAb[k], "dlAb": dlAb[k],
            "spB": spB[k], "dlB": dlB[k], "spBb": spBb[k], "dlBb": dlBb[k],
            "Wl0": Wl0, "Wr0": Wr0, "Wl1": Wl1, "Wr1": Wr1, "Wlin": Wlin_pad,
            "b0bc": b0bc, "b1bc": b1bc, "blbc": blbc,
            "iota": iota_np, "mA": mA, "mB": mB,
        })

    if TRACE:
        import sys, types, contextlib  # noqa
        if "antenv.axon_hooks" not in sys.modules:
            mod = types.ModuleType("antenv.axon_hooks")
            _h = [None]
            mod.set_axon_ntff_profile_hook = lambda h: _h.__setitem__(0, h)
            mod.get_axon_ntff_profile_hook = lambda: _h[0]
            sys.modules["antenv.axon_hooks"] = mod
            try:
                from trn_agent_boot.trn_boot import _ntff_profile_via_ctypes
                mod.set_axon_ntff_profile_hook(
                    _ntff_profile_via_ctypes("/opt/axon/libaxon_pjrt.so"))
            except Exception:
                pass

    last_err = None
    for attempt in range(4):
        try:
            res = bass_utils.run_bass_kernel_spmd(
                nc, in_maps, core_ids=list(range(NCORES)), trace=TRACE)
            break
        except Exception as e:  # noqa: BLE001
            last_err = e
            import jax
            try:
                jax.clear_caches()
            except Exception:
                pass
    else:
        raise last_err

    LAST_EXEC_NS = res.exec_time_ns

    if DEBUG_DUMPS:
        DEBUG["x1"] = [res.results[k]["dbg_x1"] for k in range(NCORES)]
        DEBUG["xm1"] = [res.results[k]["dbg_xm1"] for k in range(NCORES)]
    out = np.zeros((N2, NC_CLS), np.float32)
    for k in range(NCORES):
        r0 = k * HPC
        r1 = min((k + 1) * HPC, N2)
        if r1 > r0:
            out[r0:r1] = res.results[k]["out"][:r1 - r0, :NC_CLS]
    return out

